# revision 22
# baseline (speedup 1.0000x reference)
"""Reconstructed baseline kernel (original staged version)."""

import sys

sys.path.insert(0, "/opt/trn_rl_repo")

import numpy as np

import concourse.bacc as bacc
import concourse.bass as bass
import concourse.mybir as mybir
from concourse import bass_utils
from concourse.bass_types import AP
from concourse.tile import TileContext
from concourse.vector_clock import ScopedClock


class SlimTileContext(TileContext):
    def _drain_and_barrier(self, tick_clock, wait_clock):
        drain_inst = self.nc.gpsimd.drain()
        wait_clock.add_sem_waits(
            drain_inst.ins, ScopedClock({None: tick_clock.global_clock})
        )
        popped = self.nc._tile_sem_poison_stack.pop()
        assert popped is self._sem_poison
        self.nc.clear_and_free_semaphores(list(self.sems.allocated().values()))

B, N, L = 4096, 8192, 256
NCORES = 8
RPC = B // NCORES
P = 128
NSEG = RPC // P
W = NSEG * L
EPS = 1e-8
Alu = mybir.AluOpType
Act = mybir.ActivationFunctionType

f32 = mybir.dt.float32
i32 = mybir.dt.int32
u16 = mybir.dt.uint16


def _mkap(base: AP, off: int, dims: list[list[int]]) -> AP:
    return AP(base.tensor, base.offset + off, [list(base.ap[0])] + dims)


def _emit_sort_round(eng, src: AP, dst: AP, nseg: int, m: int, flip: bool):
    two_m = 2 * m
    nb = L // two_m
    outer = [[L, nseg]] if nseg > 1 else []

    def dims(inner_off, inner_step):
        d = outer + ([[two_m, nb]] if nb > 1 else []) + [[inner_step, m]]
        return inner_off, d

    lo_o, lo_d = dims(0, 1)
    if flip and m == 1:
        hi_o, hi_d = dims(1, 1)
    elif flip:
        hi_o, hi_d = dims(two_m - 1, -1)
    else:
        hi_o, hi_d = dims(m, 1)

    a = _mkap(src, lo_o, lo_d)
    b = _mkap(src, hi_o, hi_d)
    eng.tensor_tensor(out=_mkap(dst, lo_o, lo_d), in0=a, in1=b, op=Alu.max)
    eng.tensor_tensor(out=_mkap(dst, hi_o, hi_d), in0=a, in1=b, op=Alu.min)


def _sort_schedule():
    rounds = []
    m = 1
    while m < L:
        rounds.append((m, True))
        d = m // 2
        while d >= 1:
            rounds.append((d, False))
            d //= 2
        m *= 2
    return rounds


def _emit_sort_interleaved(eng, streams):
    rounds = _sort_schedule()
    cur = [bx for bx, _, _ in streams]
    nxt = [by for _, by, _ in streams]
    for m, flip in rounds:
        for i, (_, _, nseg) in enumerate(streams):
            _emit_sort_round(eng, cur[i][:], nxt[i][:], nseg, m, flip)
        cur, nxt = nxt, cur
    return cur


NHALF = 2
SEGS_PER_HALF = NSEG // NHALF
WH = SEGS_PER_HALF * L


def build(nc: bacc.Bacc):
    logits_d = nc.dram_tensor("logits", [RPC, N], f32, kind="ExternalInput")
    ids_d = nc.dram_tensor("ids", [RPC, L], i32, kind="ExternalInput")
    w_d = nc.dram_tensor("w", [RPC, L], f32, kind="ExternalInput")
    out_d = nc.dram_tensor("out", [P, 1], f32, kind="ExternalOutput")
    gsc_d = nc.dram_tensor("gsc", [RPC, L], f32, kind="Internal")

    with TileContext(nc) as tc:
        with (
            tc.tile_pool(name="const", bufs=1) as cpool,
            tc.tile_pool(name="work", bufs=1) as pool,
        ):
            rb = cpool.tile([P, NSEG], i32, tag="rb")
            rbi = cpool.tile([P, NSEG], i32, tag="rbi")
            for s in range(NSEG):
                nc.gpsimd.iota(
                    rb[:, s : s + 1],
                    pattern=[[0, 1]],
                    base=s * P * N,
                    channel_multiplier=N,
                )
                nc.gpsimd.iota(
                    rbi[:, s : s + 1],
                    pattern=[[0, 1]],
                    base=s * P * L,
                    channel_multiplier=L,
                )
            jc = cpool.tile([P, W], u16, tag="jc")
            nc.gpsimd.iota(
                jc[:].rearrange("p (s l) -> p s l", s=NSEG),
                pattern=[[0, NSEG], [1, L]],
                base=0,
                channel_multiplier=0,
            )

            w_sb = pool.tile([P, W], f32, tag="w")
            ids_sb = pool.tile([P, W], i32, tag="ids_sb")
            seg_src = [[L, P], [1, L]]
            for s in range(NSEG):
                dma_eng = nc.sync if s % 2 == 0 else nc.scalar
                dma_eng.dma_start(
                    out=w_sb[:, s * L : (s + 1) * L],
                    in_=AP(w_d.ap().tensor, s * P * L, seg_src),
                )
            for s in range(NSEG):
                dma_eng = nc.sync if s % 2 == 0 else nc.scalar
                dma_eng.dma_start(
                    out=ids_sb[:, s * L : (s + 1) * L],
                    in_=AP(ids_d.ap().tensor, s * P * L, seg_src),
                )

            sum_w = pool.tile([P, NSEG], f32, tag="sum_w")
            kq = pool.tile([P, W], u16, tag="kq")
            nc.vector.tensor_scalar(
                out=kq[:], in0=w_sb[:], scalar1=255.0, scalar2=None, op0=Alu.mult
            )
            scrA = pool.tile([P, W], f32, tag="scrA")
            for s in range(NSEG):
                nc.scalar.activation(
                    scrA[:, s * L : (s + 1) * L],
                    kq[:, s * L : (s + 1) * L],
                    Act.Copy,
                    accum_out=sum_w[:, s : s + 1],
                )

            HS = NSEG // 2
            WS = HS * L
            kx = pool.tile([P, W], u16, tag="kx")
            ky = pool.tile([P, W], u16, tag="ky")
            nc.vector.scalar_tensor_tensor(
                out=kx[:],
                in0=kq[:],
                scalar=256.0,
                in1=jc[:],
                op0=Alu.mult,
                op1=Alu.add,
            )
            offu = pool.tile([P, W], i32, tag="offu")
            g_u = pool.tile([P, W], f32, tag="g_u")
            nc.vector.tensor_tensor(
                out=offu[:].rearrange("p (s l) -> p s l", s=NSEG),
                in0=ids_sb[:].rearrange("p (s l) -> p s l", s=NSEG),
                in1=rb[:].to_broadcast([P, NSEG, L]),
                op=Alu.bitwise_or,
            )
            for h in range(2):
                hsl = slice(h * WS, (h + 1) * WS)
                nc.gpsimd.indirect_dma_start(
                    out=g_u[:, hsl],
                    out_offset=None,
                    in_=logits_d.ap(),
                    in_offset=bass.IndirectOffsetOnAxis(ap=offu[:, hsl], axis=1),
                )
            wb = nc.sync.dma_start(
                out=AP(gsc_d, 0, [[L, P], [P * L, NSEG], [1, L]]),
                in_=g_u[:].rearrange("p (s l) -> p s l", s=NSEG),
            )

            key_s = _emit_sort_interleaved(nc.vector, [(kx, ky, NSEG)])[0]

            off1 = pool.tile([P, W], i32, tag="off1")
            g_s = pool.tile([P, W], f32, tag="g")
            e_s = pool.tile([P, W], f32, tag="e")
            S = pool.tile([P, W], f32, tag="S")
            lse = pool.tile([P, W], f32, tag="lse")
            wqt = pool.tile([P, W], f32, tag="wqt")
            wq16 = pool.tile([P, W], u16, tag="wq16")
            j16 = pool.tile([P, W], u16, tag="j16")
            prod = pool.tile([P, W], f32, tag="prod")
            sum_wd = pool.tile([P, NSEG], f32, tag="sum_wd")

            def rev_seg(ap, s):
                return AP(
                    ap.tensor,
                    ap.offset + (s + 1) * L - 1,
                    [list(ap.ap[0]), [-1, L]],
                )

            for h in range(2):
                hsl = slice(h * WS, (h + 1) * WS)
                ks = key_s[:, hsl]

                nc.vector.tensor_scalar(
                    out=j16[:, hsl],
                    in0=ks,
                    scalar1=255,
                    scalar2=None,
                    op0=Alu.bitwise_and,
                )
                nc.vector.scalar_tensor_tensor(
                    out=off1[:, hsl].rearrange("p (s l) -> p s l", s=HS),
                    in0=j16[:, hsl].rearrange("p (s l) -> p s l", s=HS),
                    scalar=0.0,
                    in1=rbi[:, h * HS : (h + 1) * HS].to_broadcast([P, HS, L]),
                    op0=Alu.add,
                    op1=Alu.add,
                )
                ga = nc.gpsimd.indirect_dma_start(
                    out=g_s[:, hsl],
                    out_offset=None,
                    in_=gsc_d.ap(),
                    in_offset=bass.IndirectOffsetOnAxis(ap=off1[:, hsl], axis=1),
                )
                bass._add_dep_helper(
                    ga.ins, wb.ins, sync=True, reason="gather reads gsc scratch"
                )
                nc.scalar.activation(e_s[:, hsl], g_s[:, hsl], Act.Exp)
                for s in range(h * HS, (h + 1) * HS):
                    nc.vector.tensor_tensor_scan(
                        out=rev_seg(S[:], s),
                        data0=rev_seg(e_s[:], s),
                        data1=rev_seg(e_s[:], s),
                        initial=0.0,
                        op0=Alu.add,
                        op1=Alu.bypass,
                    )
                nc.vector.tensor_scalar(
                    out=wq16[:, hsl],
                    in0=ks,
                    scalar1=8,
                    scalar2=None,
                    op0=Alu.logical_shift_right,
                )
                nc.vector.tensor_copy(out=wqt[:, hsl], in_=wq16[:, hsl])

            for h in range(2):
                hsl = slice(h * WS, (h + 1) * WS)
                nc.scalar.activation(lse[:, hsl], S[:, hsl], Act.Ln)
                nc.vector.tensor_tensor(
                    out=lse[:, hsl],
                    in0=lse[:, hsl],
                    in1=g_s[:, hsl],
                    op=Alu.subtract,
                )
                nc.vector.tensor_tensor(
                    out=prod[:, hsl],
                    in0=wqt[:, hsl],
                    in1=lse[:, hsl],
                    op=Alu.mult,
                )
                nc.vector.tensor_reduce(
                    out=sum_wd[:, h * HS : (h + 1) * HS],
                    in_=prod[:, hsl].rearrange("p (s l) -> p s l", s=HS),
                    axis=mybir.AxisListType.X,
                    op=Alu.add,
                )

            nc.vector.tensor_scalar(
                out=sum_w[:], in0=sum_w[:], scalar1=EPS, scalar2=None, op0=Alu.max
            )
            rcp = pool.tile([P, NSEG], f32, tag="rcp")
            nc.vector.reciprocal(out=rcp[:], in_=sum_w[:])
            nc.vector.tensor_tensor(
                out=sum_wd[:], in0=sum_wd[:], in1=rcp[:], op=Alu.mult
            )
            acc = pool.tile([P, 1], f32, tag="acc")
            nc.vector.tensor_reduce(
                out=acc[:], in_=sum_wd[:], axis=mybir.AxisListType.X, op=Alu.add
            )
            nc.sync.dma_start(out=out_d.ap(), in_=acc[:])

    nc.compile()
    return nc


_CACHED = None


def _get_nc():
    global _CACHED
    if _CACHED is None:
        nc = bacc.Bacc("TRN2", debug=False, num_devices=NCORES)
        _CACHED = build(nc)
    return _CACHED


def kernel(logits, positive_ids, positive_weights, _trace=False):
    logits = np.ascontiguousarray(np.asarray(logits, dtype=np.float32))
    ids = np.ascontiguousarray(np.asarray(positive_ids, dtype=np.int32))
    w = np.ascontiguousarray(np.asarray(positive_weights, dtype=np.float32))
    assert logits.shape == (B, N) and ids.shape == (B, L) and w.shape == (B, L)

    nc = _get_nc()
    in_maps = [
        {
            "logits": logits[c * RPC : (c + 1) * RPC],
            "ids": ids[c * RPC : (c + 1) * RPC],
            "w": w[c * RPC : (c + 1) * RPC],
        }
        for c in range(NCORES)
    ]
    res = bass_utils.run_bass_kernel_spmd(
        nc, in_maps, core_ids=list(range(NCORES)), trace=_trace
    )
    total = np.float64(0.0)
    for r in res.results:
        total += np.float64(r["out"].sum())
    out = np.array(total / B, dtype=np.float32)
    if _trace:
        return out, res
    return out


# revision 23
# speedup vs baseline: 1.0448x; 1.0448x over previous
"""Reconstructed baseline kernel (original staged version)."""

import sys

sys.path.insert(0, "/opt/trn_rl_repo")

import numpy as np

import concourse.bacc as bacc
import concourse.bass as bass
import concourse.mybir as mybir
from concourse import bass_utils
from concourse.bass_types import AP
from concourse.tile import TileContext
from concourse.vector_clock import ScopedClock


class SlimTileContext(TileContext):
    def _drain_and_barrier(self, tick_clock, wait_clock):
        drain_inst = self.nc.gpsimd.drain()
        wait_clock.add_sem_waits(
            drain_inst.ins, ScopedClock({None: tick_clock.global_clock})
        )
        popped = self.nc._tile_sem_poison_stack.pop()
        assert popped is self._sem_poison
        self.nc.clear_and_free_semaphores(list(self.sems.allocated().values()))

B, N, L = 4096, 8192, 256
NCORES = 8
RPC = B // NCORES
P = 128
NSEG = RPC // P
W = NSEG * L
EPS = 1e-8
Alu = mybir.AluOpType
Act = mybir.ActivationFunctionType

f32 = mybir.dt.float32
i32 = mybir.dt.int32
u16 = mybir.dt.uint16


def _mkap(base: AP, off: int, dims: list[list[int]]) -> AP:
    return AP(base.tensor, base.offset + off, [list(base.ap[0])] + dims)


def _emit_sort_round(eng, src: AP, dst: AP, nseg: int, m: int, flip: bool):
    two_m = 2 * m
    nb = L // two_m
    outer = [[L, nseg]] if nseg > 1 else []

    def dims(inner_off, inner_step):
        d = outer + ([[two_m, nb]] if nb > 1 else []) + [[inner_step, m]]
        return inner_off, d

    lo_o, lo_d = dims(0, 1)
    if flip and m == 1:
        hi_o, hi_d = dims(1, 1)
    elif flip:
        hi_o, hi_d = dims(two_m - 1, -1)
    else:
        hi_o, hi_d = dims(m, 1)

    a = _mkap(src, lo_o, lo_d)
    b = _mkap(src, hi_o, hi_d)
    eng.tensor_tensor(out=_mkap(dst, lo_o, lo_d), in0=a, in1=b, op=Alu.max)
    eng.tensor_tensor(out=_mkap(dst, hi_o, hi_d), in0=a, in1=b, op=Alu.min)


def _sort_schedule():
    rounds = []
    m = 1
    while m < L:
        rounds.append((m, True))
        d = m // 2
        while d >= 1:
            rounds.append((d, False))
            d //= 2
        m *= 2
    return rounds


def _emit_sort_interleaved(eng, streams):
    rounds = _sort_schedule()
    cur = [bx for bx, _, _ in streams]
    nxt = [by for _, by, _ in streams]
    for m, flip in rounds:
        for i, (_, _, nseg) in enumerate(streams):
            _emit_sort_round(eng, cur[i][:], nxt[i][:], nseg, m, flip)
        cur, nxt = nxt, cur
    return cur


NHALF = 2
SEGS_PER_HALF = NSEG // NHALF
WH = SEGS_PER_HALF * L


def build(nc: bacc.Bacc):
    logits_d = nc.dram_tensor("logits", [RPC, N], f32, kind="ExternalInput")
    ids_d = nc.dram_tensor("ids", [RPC, L], i32, kind="ExternalInput")
    w_d = nc.dram_tensor("w", [RPC, L], f32, kind="ExternalInput")
    out_d = nc.dram_tensor("out", [P, 1], f32, kind="ExternalOutput")
    gsc_d = nc.dram_tensor("gsc", [RPC, L], f32, kind="Internal")

    with TileContext(nc) as tc:
        with (
            tc.tile_pool(name="const", bufs=1) as cpool,
            tc.tile_pool(name="work", bufs=1) as pool,
        ):
            rb = cpool.tile([P, NSEG], i32, tag="rb")
            rbi = cpool.tile([P, NSEG], i32, tag="rbi")
            for s in range(NSEG):
                nc.gpsimd.iota(
                    rb[:, s : s + 1],
                    pattern=[[0, 1]],
                    base=s * P * N,
                    channel_multiplier=N,
                )
                nc.gpsimd.iota(
                    rbi[:, s : s + 1],
                    pattern=[[0, 1]],
                    base=s * P * L,
                    channel_multiplier=L,
                )
            jc = cpool.tile([P, W], u16, tag="jc")
            nc.gpsimd.iota(
                jc[:].rearrange("p (s l) -> p s l", s=NSEG),
                pattern=[[0, NSEG], [1, L]],
                base=0,
                channel_multiplier=0,
            )

            w_sb = pool.tile([P, W], f32, tag="w")
            ids_sb = pool.tile([P, W], i32, tag="ids_sb")
            HSD = NSEG // 2
            WSD = HSD * L
            for h, dma_eng in ((0, nc.sync), (1, nc.scalar)):
                dma_eng.dma_start(
                    out=w_sb[:, h * WSD : (h + 1) * WSD].rearrange(
                        "p (s l) -> p s l", s=HSD
                    ),
                    in_=AP(w_d.ap().tensor, h * WSD * P, [[L, P], [P * L, HSD], [1, L]]),
                )
            for h, dma_eng in ((0, nc.sync), (1, nc.scalar)):
                dma_eng.dma_start(
                    out=ids_sb[:, h * WSD : (h + 1) * WSD].rearrange(
                        "p (s l) -> p s l", s=HSD
                    ),
                    in_=AP(ids_d.ap().tensor, h * WSD * P, [[L, P], [P * L, HSD], [1, L]]),
                )

            sum_w = pool.tile([P, NSEG], f32, tag="sum_w")
            kq = pool.tile([P, W], u16, tag="kq")
            nc.vector.tensor_scalar(
                out=kq[:], in0=w_sb[:], scalar1=255.0, scalar2=None, op0=Alu.mult
            )
            scrA = pool.tile([P, W], f32, tag="scrA")
            for s in range(NSEG):
                nc.scalar.activation(
                    scrA[:, s * L : (s + 1) * L],
                    kq[:, s * L : (s + 1) * L],
                    Act.Copy,
                    accum_out=sum_w[:, s : s + 1],
                )

            HS = NSEG // 2
            WS = HS * L
            kx = pool.tile([P, W], u16, tag="kx")
            ky = pool.tile([P, W], u16, tag="ky")
            nc.vector.scalar_tensor_tensor(
                out=kx[:],
                in0=kq[:],
                scalar=256.0,
                in1=jc[:],
                op0=Alu.mult,
                op1=Alu.add,
            )
            offu = pool.tile([P, W], i32, tag="offu")
            g_u = pool.tile([P, W], f32, tag="g_u")
            nc.vector.tensor_tensor(
                out=offu[:].rearrange("p (s l) -> p s l", s=NSEG),
                in0=ids_sb[:].rearrange("p (s l) -> p s l", s=NSEG),
                in1=rb[:].to_broadcast([P, NSEG, L]),
                op=Alu.bitwise_or,
            )
            for h in range(2):
                hsl = slice(h * WS, (h + 1) * WS)
                nc.gpsimd.indirect_dma_start(
                    out=g_u[:, hsl],
                    out_offset=None,
                    in_=logits_d.ap(),
                    in_offset=bass.IndirectOffsetOnAxis(ap=offu[:, hsl], axis=1),
                )
            wb = nc.sync.dma_start(
                out=AP(gsc_d, 0, [[L, P], [P * L, NSEG], [1, L]]),
                in_=g_u[:].rearrange("p (s l) -> p s l", s=NSEG),
            )

            key_s = _emit_sort_interleaved(nc.vector, [(kx, ky, NSEG)])[0]

            off1 = pool.tile([P, W], i32, tag="off1")
            g_s = pool.tile([P, W], f32, tag="g")
            e_s = pool.tile([P, W], f32, tag="e")
            S = pool.tile([P, W], f32, tag="S")
            lse = pool.tile([P, W], f32, tag="lse")
            wqt = pool.tile([P, W], f32, tag="wqt")
            wq16 = pool.tile([P, W], u16, tag="wq16")
            j16 = pool.tile([P, W], u16, tag="j16")
            prod = pool.tile([P, W], f32, tag="prod")
            sum_wd = pool.tile([P, NSEG], f32, tag="sum_wd")

            def rev_seg(ap, s):
                return AP(
                    ap.tensor,
                    ap.offset + (s + 1) * L - 1,
                    [list(ap.ap[0]), [-1, L]],
                )

            for h in range(2):
                hsl = slice(h * WS, (h + 1) * WS)
                ks = key_s[:, hsl]

                nc.vector.tensor_scalar(
                    out=j16[:, hsl],
                    in0=ks,
                    scalar1=255,
                    scalar2=None,
                    op0=Alu.bitwise_and,
                )
                nc.vector.scalar_tensor_tensor(
                    out=off1[:, hsl].rearrange("p (s l) -> p s l", s=HS),
                    in0=j16[:, hsl].rearrange("p (s l) -> p s l", s=HS),
                    scalar=0.0,
                    in1=rbi[:, h * HS : (h + 1) * HS].to_broadcast([P, HS, L]),
                    op0=Alu.add,
                    op1=Alu.add,
                )
                ga = nc.gpsimd.indirect_dma_start(
                    out=g_s[:, hsl],
                    out_offset=None,
                    in_=gsc_d.ap(),
                    in_offset=bass.IndirectOffsetOnAxis(ap=off1[:, hsl], axis=1),
                )
                bass._add_dep_helper(
                    ga.ins, wb.ins, sync=True, reason="gather reads gsc scratch"
                )
                nc.scalar.activation(e_s[:, hsl], g_s[:, hsl], Act.Exp)
                for s in range(h * HS, (h + 1) * HS):
                    nc.vector.tensor_tensor_scan(
                        out=rev_seg(S[:], s),
                        data0=rev_seg(e_s[:], s),
                        data1=rev_seg(e_s[:], s),
                        initial=0.0,
                        op0=Alu.add,
                        op1=Alu.bypass,
                    )
                nc.vector.tensor_scalar(
                    out=wq16[:, hsl],
                    in0=ks,
                    scalar1=8,
                    scalar2=None,
                    op0=Alu.logical_shift_right,
                )
                nc.vector.tensor_copy(out=wqt[:, hsl], in_=wq16[:, hsl])

            for h in range(2):
                hsl = slice(h * WS, (h + 1) * WS)
                nc.scalar.activation(lse[:, hsl], S[:, hsl], Act.Ln)
                nc.vector.tensor_tensor(
                    out=lse[:, hsl],
                    in0=lse[:, hsl],
                    in1=g_s[:, hsl],
                    op=Alu.subtract,
                )
                nc.vector.tensor_tensor(
                    out=prod[:, hsl],
                    in0=wqt[:, hsl],
                    in1=lse[:, hsl],
                    op=Alu.mult,
                )
                nc.vector.tensor_reduce(
                    out=sum_wd[:, h * HS : (h + 1) * HS],
                    in_=prod[:, hsl].rearrange("p (s l) -> p s l", s=HS),
                    axis=mybir.AxisListType.X,
                    op=Alu.add,
                )

            nc.vector.tensor_scalar(
                out=sum_w[:], in0=sum_w[:], scalar1=EPS, scalar2=None, op0=Alu.max
            )
            rcp = pool.tile([P, NSEG], f32, tag="rcp")
            nc.vector.reciprocal(out=rcp[:], in_=sum_w[:])
            nc.vector.tensor_tensor(
                out=sum_wd[:], in0=sum_wd[:], in1=rcp[:], op=Alu.mult
            )
            acc = pool.tile([P, 1], f32, tag="acc")
            nc.vector.tensor_reduce(
                out=acc[:], in_=sum_wd[:], axis=mybir.AxisListType.X, op=Alu.add
            )
            nc.sync.dma_start(out=out_d.ap(), in_=acc[:])

    nc.compile()
    return nc


_CACHED = None


def _get_nc():
    global _CACHED
    if _CACHED is None:
        nc = bacc.Bacc("TRN2", debug=False, num_devices=NCORES)
        _CACHED = build(nc)
    return _CACHED


def kernel(logits, positive_ids, positive_weights, _trace=False):
    logits = np.ascontiguousarray(np.asarray(logits, dtype=np.float32))
    ids = np.ascontiguousarray(np.asarray(positive_ids, dtype=np.int32))
    w = np.ascontiguousarray(np.asarray(positive_weights, dtype=np.float32))
    assert logits.shape == (B, N) and ids.shape == (B, L) and w.shape == (B, L)

    nc = _get_nc()
    in_maps = [
        {
            "logits": logits[c * RPC : (c + 1) * RPC],
            "ids": ids[c * RPC : (c + 1) * RPC],
            "w": w[c * RPC : (c + 1) * RPC],
        }
        for c in range(NCORES)
    ]
    res = bass_utils.run_bass_kernel_spmd(
        nc, in_maps, core_ids=list(range(NCORES)), trace=_trace
    )
    total = np.float64(0.0)
    for r in res.results:
        total += np.float64(r["out"].sum())
    out = np.array(total / B, dtype=np.float32)
    if _trace:
        return out, res
    return out


# revision 27
# speedup vs baseline: 1.1154x; 1.0675x over previous
"""Reconstructed baseline kernel (original staged version)."""

import sys

sys.path.insert(0, "/opt/trn_rl_repo")

import numpy as np

import concourse.bacc as bacc
import concourse.bass as bass
import concourse.mybir as mybir
from concourse import bass_utils
from concourse.bass_types import AP
from concourse.tile import TileContext
from concourse.vector_clock import ScopedClock


class SlimTileContext(TileContext):
    def _drain_and_barrier(self, tick_clock, wait_clock):
        drain_inst = self.nc.gpsimd.drain()
        wait_clock.add_sem_waits(
            drain_inst.ins, ScopedClock({None: tick_clock.global_clock})
        )
        popped = self.nc._tile_sem_poison_stack.pop()
        assert popped is self._sem_poison
        self.nc.clear_and_free_semaphores(list(self.sems.allocated().values()))

B, N, L = 4096, 8192, 256
NCORES = 8
RPC = B // NCORES
P = 128
NSEG = RPC // P
W = NSEG * L
EPS = 1e-8
Alu = mybir.AluOpType
Act = mybir.ActivationFunctionType

f32 = mybir.dt.float32
i32 = mybir.dt.int32
u16 = mybir.dt.uint16


def _mkap(base: AP, off: int, dims: list[list[int]]) -> AP:
    return AP(base.tensor, base.offset + off, [list(base.ap[0])] + dims)


def _emit_sort_round(eng, src: AP, dst: AP, nseg: int, m: int, flip: bool):
    two_m = 2 * m
    nb = L // two_m
    outer = [[L, nseg]] if nseg > 1 else []

    def dims(inner_off, inner_step):
        d = outer + ([[two_m, nb]] if nb > 1 else []) + [[inner_step, m]]
        return inner_off, d

    lo_o, lo_d = dims(0, 1)
    if flip and m == 1:
        hi_o, hi_d = dims(1, 1)
    elif flip:
        hi_o, hi_d = dims(two_m - 1, -1)
    else:
        hi_o, hi_d = dims(m, 1)

    a = _mkap(src, lo_o, lo_d)
    b = _mkap(src, hi_o, hi_d)
    eng.tensor_tensor(out=_mkap(dst, lo_o, lo_d), in0=a, in1=b, op=Alu.max)
    eng.tensor_tensor(out=_mkap(dst, hi_o, hi_d), in0=a, in1=b, op=Alu.min)


def _sort_schedule():
    rounds = []
    m = 1
    while m < L:
        rounds.append((m, True))
        d = m // 2
        while d >= 1:
            rounds.append((d, False))
            d //= 2
        m *= 2
    return rounds


def _emit_sort_interleaved(eng, streams):
    rounds = _sort_schedule()
    cur = [bx for bx, _, _ in streams]
    nxt = [by for _, by, _ in streams]
    for m, flip in rounds:
        for i, (_, _, nseg) in enumerate(streams):
            _emit_sort_round(eng, cur[i][:], nxt[i][:], nseg, m, flip)
        cur, nxt = nxt, cur
    return cur


NHALF = 2
SEGS_PER_HALF = NSEG // NHALF
WH = SEGS_PER_HALF * L


def build(nc: bacc.Bacc):
    logits_d = nc.dram_tensor("logits", [RPC, N], f32, kind="ExternalInput")
    ids_d = nc.dram_tensor("ids", [RPC, L], i32, kind="ExternalInput")
    w_d = nc.dram_tensor("w", [RPC, L], f32, kind="ExternalInput")
    out_d = nc.dram_tensor("out", [1, 1], f32, kind="ExternalOutput")
    gsc_d = nc.dram_tensor("gsc", [RPC, L], f32, kind="Internal")

    with TileContext(nc) as tc:
        with (
            tc.tile_pool(name="const", bufs=1) as cpool,
            tc.tile_pool(name="work", bufs=1) as pool,
            tc.tile_pool(name="ps", bufs=1, space="PSUM") as ppool,
        ):
            rb = cpool.tile([P, NSEG], i32, tag="rb")
            rbi = cpool.tile([P, NSEG], i32, tag="rbi")
            for s in range(NSEG):
                nc.gpsimd.iota(
                    rb[:, s : s + 1],
                    pattern=[[0, 1]],
                    base=s * P * N,
                    channel_multiplier=N,
                )
                nc.gpsimd.iota(
                    rbi[:, s : s + 1],
                    pattern=[[0, 1]],
                    base=s * P * L,
                    channel_multiplier=L,
                )
            jc = cpool.tile([P, W], u16, tag="jc")
            nc.gpsimd.iota(
                jc[:].rearrange("p (s l) -> p s l", s=NSEG),
                pattern=[[0, NSEG], [1, L]],
                base=0,
                channel_multiplier=0,
            )
            ones = cpool.tile([P, 1], f32, tag="ones")
            nc.vector.memset(ones[:], 1.0)

            w_sb = pool.tile([P, W], f32, tag="w")
            ids_sb = pool.tile([P, W], i32, tag="ids_sb")
            HSD = NSEG // 2
            WSD = HSD * L
            for h, dma_eng in ((0, nc.sync), (1, nc.scalar)):
                dma_eng.dma_start(
                    out=w_sb[:, h * WSD : (h + 1) * WSD].rearrange(
                        "p (s l) -> p s l", s=HSD
                    ),
                    in_=AP(w_d.ap().tensor, h * WSD * P, [[L, P], [P * L, HSD], [1, L]]),
                )
            for h, dma_eng in ((0, nc.sync), (1, nc.scalar)):
                dma_eng.dma_start(
                    out=ids_sb[:, h * WSD : (h + 1) * WSD].rearrange(
                        "p (s l) -> p s l", s=HSD
                    ),
                    in_=AP(ids_d.ap().tensor, h * WSD * P, [[L, P], [P * L, HSD], [1, L]]),
                )

            sum_w = pool.tile([P, NSEG], f32, tag="sum_w")
            kq = pool.tile([P, W], u16, tag="kq")
            nc.vector.tensor_scalar(
                out=kq[:], in0=w_sb[:], scalar1=255.0, scalar2=None, op0=Alu.mult
            )
            scrA = pool.tile([P, W], f32, tag="scrA")
            for s in range(NSEG):
                nc.scalar.activation(
                    scrA[:, s * L : (s + 1) * L],
                    kq[:, s * L : (s + 1) * L],
                    Act.Copy,
                    accum_out=sum_w[:, s : s + 1],
                )

            HS = NSEG // 2
            WS = HS * L
            kx = pool.tile([P, W], u16, tag="kx")
            ky = pool.tile([P, W], u16, tag="ky")
            nc.vector.scalar_tensor_tensor(
                out=kx[:],
                in0=kq[:],
                scalar=256.0,
                in1=jc[:],
                op0=Alu.mult,
                op1=Alu.add,
            )
            offu = pool.tile([P, W], i32, tag="offu")
            g_u = pool.tile([P, W], f32, tag="g_u")
            nc.vector.tensor_tensor(
                out=offu[:].rearrange("p (s l) -> p s l", s=NSEG),
                in0=ids_sb[:].rearrange("p (s l) -> p s l", s=NSEG),
                in1=rb[:].to_broadcast([P, NSEG, L]),
                op=Alu.bitwise_or,
            )
            for h in range(2):
                hsl = slice(h * WS, (h + 1) * WS)
                nc.gpsimd.indirect_dma_start(
                    out=g_u[:, hsl],
                    out_offset=None,
                    in_=logits_d.ap(),
                    in_offset=bass.IndirectOffsetOnAxis(ap=offu[:, hsl], axis=1),
                )
            wb = nc.sync.dma_start(
                out=AP(gsc_d, 0, [[L, P], [P * L, NSEG], [1, L]]),
                in_=g_u[:].rearrange("p (s l) -> p s l", s=NSEG),
            )

            key_s = _emit_sort_interleaved(nc.vector, [(kx, ky, NSEG)])[0]

            off1 = pool.tile([P, W], i32, tag="off1")
            g_s = pool.tile([P, W], f32, tag="g")
            e_s = pool.tile([P, W], f32, tag="e")
            S = pool.tile([P, W], f32, tag="S")
            lse = pool.tile([P, W], f32, tag="lse")
            wqt = pool.tile([P, W], f32, tag="wqt")
            wq16 = pool.tile([P, W], u16, tag="wq16")
            j16 = pool.tile([P, W], u16, tag="j16")
            prod = pool.tile([P, W], f32, tag="prod")
            sum_wd = pool.tile([P, NSEG], f32, tag="sum_wd")

            def rev_seg(ap, s):
                return AP(
                    ap.tensor,
                    ap.offset + (s + 1) * L - 1,
                    [list(ap.ap[0]), [-1, L]],
                )

            for h in range(2):
                hsl = slice(h * WS, (h + 1) * WS)
                ks = key_s[:, hsl]

                nc.vector.tensor_scalar(
                    out=j16[:, hsl],
                    in0=ks,
                    scalar1=255,
                    scalar2=None,
                    op0=Alu.bitwise_and,
                )
                nc.vector.scalar_tensor_tensor(
                    out=off1[:, hsl].rearrange("p (s l) -> p s l", s=HS),
                    in0=j16[:, hsl].rearrange("p (s l) -> p s l", s=HS),
                    scalar=0.0,
                    in1=rbi[:, h * HS : (h + 1) * HS].to_broadcast([P, HS, L]),
                    op0=Alu.add,
                    op1=Alu.add,
                )
                ga = nc.gpsimd.indirect_dma_start(
                    out=g_s[:, hsl],
                    out_offset=None,
                    in_=gsc_d.ap(),
                    in_offset=bass.IndirectOffsetOnAxis(ap=off1[:, hsl], axis=1),
                )
                bass._add_dep_helper(
                    ga.ins, wb.ins, sync=True, reason="gather reads gsc scratch"
                )
                nc.scalar.activation(e_s[:, hsl], g_s[:, hsl], Act.Exp)
                for s in range(h * HS, (h + 1) * HS):
                    nc.vector.tensor_tensor_scan(
                        out=rev_seg(S[:], s),
                        data0=rev_seg(e_s[:], s),
                        data1=rev_seg(e_s[:], s),
                        initial=0.0,
                        op0=Alu.add,
                        op1=Alu.bypass,
                    )
                nc.vector.tensor_scalar(
                    out=wq16[:, hsl],
                    in0=ks,
                    scalar1=8,
                    scalar2=None,
                    op0=Alu.logical_shift_right,
                )
                nc.vector.tensor_copy(out=wqt[:, hsl], in_=wq16[:, hsl])

            for h in range(2):
                hsl = slice(h * WS, (h + 1) * WS)
                nc.scalar.activation(lse[:, hsl], S[:, hsl], Act.Ln)
                nc.vector.tensor_tensor(
                    out=lse[:, hsl],
                    in0=lse[:, hsl],
                    in1=g_s[:, hsl],
                    op=Alu.subtract,
                )
                nc.vector.tensor_tensor(
                    out=prod[:, hsl],
                    in0=wqt[:, hsl],
                    in1=lse[:, hsl],
                    op=Alu.mult,
                )
                nc.vector.tensor_reduce(
                    out=sum_wd[:, h * HS : (h + 1) * HS],
                    in_=prod[:, hsl].rearrange("p (s l) -> p s l", s=HS),
                    axis=mybir.AxisListType.X,
                    op=Alu.add,
                )

            nc.vector.tensor_scalar(
                out=sum_w[:], in0=sum_w[:], scalar1=EPS, scalar2=None, op0=Alu.max
            )
            rcp = pool.tile([P, NSEG], f32, tag="rcp")
            nc.vector.reciprocal(out=rcp[:], in_=sum_w[:])
            nc.vector.tensor_tensor(
                out=sum_wd[:], in0=sum_wd[:], in1=rcp[:], op=Alu.mult
            )
            acc = pool.tile([P, 1], f32, tag="acc")
            nc.vector.tensor_reduce(
                out=acc[:], in_=sum_wd[:], axis=mybir.AxisListType.X, op=Alu.add
            )
            ps = ppool.tile([1, 1], f32, tag="ps")
            nc.tensor.matmul(ps[:], acc[:], ones[:], start=True, stop=True)
            res = pool.tile([1, 1], f32, tag="res")
            nc.vector.tensor_copy(out=res[:], in_=ps[:])
            nc.sync.dma_start(out=out_d.ap(), in_=res[:])

    nc.compile()
    return nc


_CACHED = None


def _get_nc():
    global _CACHED
    if _CACHED is None:
        nc = bacc.Bacc("TRN2", debug=False, num_devices=NCORES)
        _CACHED = build(nc)
    return _CACHED


def kernel(logits, positive_ids, positive_weights, _trace=False):
    logits = np.ascontiguousarray(np.asarray(logits, dtype=np.float32))
    ids = np.ascontiguousarray(np.asarray(positive_ids, dtype=np.int32))
    w = np.ascontiguousarray(np.asarray(positive_weights, dtype=np.float32))
    assert logits.shape == (B, N) and ids.shape == (B, L) and w.shape == (B, L)

    nc = _get_nc()
    in_maps = [
        {
            "logits": logits[c * RPC : (c + 1) * RPC],
            "ids": ids[c * RPC : (c + 1) * RPC],
            "w": w[c * RPC : (c + 1) * RPC],
        }
        for c in range(NCORES)
    ]
    res = bass_utils.run_bass_kernel_spmd(
        nc, in_maps, core_ids=list(range(NCORES)), trace=_trace
    )
    total = np.float64(0.0)
    for r in res.results:
        total += np.float64(r["out"][0, 0])
    out = np.array(total / B, dtype=np.float32)
    if _trace:
        return out, res
    return out
